# revision 38
# baseline (speedup 1.0000x reference)
"""AttentionPerformer Trainium2 kernel.

Strategy: data-parallel over batch B=8 -> one NeuronCore per batch element.
Per core, everything is computed in a "feature-major" (transposed) layout so
that the big matmuls keep their stationary weights resident and the Performer
random-feature exponent (wtx - 0.5*|k|^2) comes directly out of the PE via a
fused contraction over [k ; k^2].

Math (per b, h; eps folded: dropping both 1/sqrt(M) factors rescales
numerator and denominator by M, so eps becomes M*eps):
    kq^T = W_kq^T @ x^T            (feature-major, per-head k/q stacked 64+64)
    e_k  = w_h @ k - 0.5|k|^2      (one PE pass over [k^T ; (k^T)^2])
    kp = exp(e_k), qp = exp(e_q)
    kptv[m, n] = sum_t kp[t,m] v[t,n] ;  kp_sum[m] = sum_t kp[t,m]
    y^T_h = (kptv_aug^T @ qp^T) ; D = kp_sum . qp
    out^T = proj_w^T.T @ (y^T / (D + M*eps)) + proj_b

dtypes: fp32r (TRN2 full-rate fp32, ~1.5e-4) on the exponent-critical path,
bf16 on the kp/v/qp value path (errors average over the M=32 contraction).
"""
import sys
sys.path.insert(0, '/opt/trn_rl_repo')

import numpy as np
import ml_dtypes

B, N, C = 8, 4096, 768
H, HD, M = 12, 64, 32
T = 512                 # t-tile size
NT = N // T             # 8 tiles
EPS_EFF = float(M) * 1e-8

_CACHE = {}
TRACE = False
LAST_EXEC_NS = None


def _build():
    import concourse.bass as bass
    import concourse.tile as tile
    from concourse import bacc, mybir

    f32 = mybir.dt.float32
    f32r = mybir.dt.float32r
    bf16 = mybir.dt.bfloat16
    ADD = mybir.AluOpType.add
    MULT = mybir.AluOpType.mult
    EXP = mybir.ActivationFunctionType.Exp
    SQUARE = mybir.ActivationFunctionType.Square
    RECIP = mybir.ActivationFunctionType.Reciprocal

    nc = bacc.Bacc()

    xT = nc.dram_tensor("xT", [C, N], f32, kind="ExternalInput")
    wkq = nc.dram_tensor("wkq", [C, 2 * C], f32, kind="ExternalInput")
    wv = nc.dram_tensor("wv", [C, C], f32, kind="ExternalInput")
    prmw = nc.dram_tensor("prmw", [128, H * M], f32, kind="ExternalInput")
    kqb = nc.dram_tensor("kqb", [128, 2 * H], f32, kind="ExternalInput")
    vbr = nc.dram_tensor("vbr", [128, C], f32, kind="ExternalInput")
    pwT = nc.dram_tensor("pwT", [C, C], f32, kind="ExternalInput")
    pb = nc.dram_tensor("pb", [128, 6], f32, kind="ExternalInput")
    identb = nc.dram_tensor("identb", [128, 128], mybir.dt.bfloat16,
                            kind="ExternalInput")
    yT = nc.dram_tensor("yT", [C, N], f32, kind="ExternalOutput")

    with tile.TileContext(nc) as tc:
        import contextlib
        with contextlib.ExitStack() as ctx:
            const = ctx.enter_context(tc.tile_pool(name="const", bufs=1))

            # ---- resident constants ----
            wkq_sb = []
            wv_sb = []
            pwT_sb = []
            for c in range(6):
                t_ = const.tile([128, 2 * C], f32r, tag=f"wkq{c}")
                nc.sync.dma_start(t_, wkq[c * 128:(c + 1) * 128, :].bitcast(f32r))
                wkq_sb.append(t_)
            for c in range(6):
                t_ = const.tile([128, C], f32r, tag=f"wv{c}")
                nc.sync.dma_start(t_, wv[c * 128:(c + 1) * 128, :].bitcast(f32r))
                wv_sb.append(t_)
            prmw_sb = const.tile([128, H, M], f32r, tag="prmw")
            nc.sync.dma_start(prmw_sb, prmw[:].bitcast(f32r).rearrange(
                "p (h m) -> p h m", h=H))
            kqb_sb = const.tile([128, 2 * H], f32, tag="kqb")
            nc.sync.dma_start(kqb_sb, kqb[:])
            vbr_sb = const.tile([128, C], f32, tag="vbr")
            nc.sync.dma_start(vbr_sb, vbr[:])
            ident_sb = const.tile([128, 128], bf16, tag="identb")
            nc.sync.dma_start(ident_sb, identb[:])

            # resident accumulators / state
            qp_pack = const.tile([128, 3, N], bf16, tag="qp_pack")
            acc = const.tile([128, 3, M * 2 + 1], f32, tag="acc")
            nc.vector.memset(acc, 0.0)
            # acc layout: [128, g, 65]; head h = 4g+gi lives at partitions
            # 32gi..32gi+32: cols 0:64 = kptv^T, col 64 = kp_sum

            # ================= PASS 1 =================
            with tc.tile_pool(name="xt", bufs=3) as xtp, \
                 tc.tile_pool(name="kq", bufs=4) as kqp, \
                 tc.tile_pool(name="sq", bufs=4) as sqp, \
                 tc.tile_pool(name="kpt", bufs=3) as kptp, \
                 tc.tile_pool(name="kpn", bufs=4) as kpnp, \
                 tc.tile_pool(name="vaug", bufs=6) as vap, \
                 tc.tile_pool(name="ps_kq", bufs=2, space="PSUM") as ps_kq, \
                 tc.tile_pool(name="ps_v", bufs=1, space="PSUM") as ps_v, \
                 tc.tile_pool(name="ps_prm", bufs=2, space="PSUM") as ps_prm, \
                 tc.tile_pool(name="ps_kptv", bufs=1, space="PSUM") as ps_kptv, \
                 tc.tile_pool(name="ps_tr", bufs=1, space="PSUM") as ps_tr:

                for it in range(NT):
                    t0 = it * T
                    xt = xtp.tile([128, 6, T], f32r, tag="xt")
                    for c in range(6):
                        nc.sync.dma_start(
                            xt[:, c, :],
                            xT[c * 128:(c + 1) * 128, t0:t0 + T].bitcast(f32r))

                    # ---- k/q head tiles, v-sub groups interleaved ----
                    vaugs = []

                    def emit_v(sub):
                        psv = ps_v.tile([128, C], f32, tag="psv")
                        for c in range(6):
                            lhs = xt[:, c, sub * 128:(sub + 1) * 128]
                            nc.tensor.matmul(psv[:, 0:512], lhs,
                                             wv_sb[c][:, 0:512],
                                             start=(c == 0), stop=(c == 5))
                            nc.tensor.matmul(psv[:, 512:768], lhs,
                                             wv_sb[c][:, 512:768],
                                             start=(c == 0), stop=(c == 5))
                        va = vap.tile([128, H, HD + 1], bf16, tag="vaug")
                        nc.vector.tensor_tensor(
                            va[:, :, 0:HD],
                            psv[:].rearrange("p (h n) -> p h n", h=H),
                            vbr_sb[:].rearrange("p (h n) -> p h n", h=H), ADD)
                        nc.vector.memset(va[:, :, HD:HD + 1], 1.0)
                        vaugs.append(va)

                    kpts = []
                    for h in range(12):
                        g, gi = h // 4, h % 4
                        if h % 3 == 0:
                            emit_v(h // 3)
                        pkq = ps_kq.tile([128, T], f32, tag="pkq")
                        for c in range(6):
                            nc.tensor.matmul(
                                pkq, wkq_sb[c][:, h * 128:(h + 1) * 128],
                                xt[:, c, :], start=(c == 0), stop=(c == 5))
                        biask = kqb_sb[0:64, h:h + 1]
                        biasq = kqb_sb[64:128, h:h + 1]
                        # ksq = [k+b ; (k+b)^2], qsq = [q+b ; (q+b)^2]
                        ksq = kqp.tile([128, T], f32r, tag="kq")
                        nc.scalar.activation(ksq[0:64, :], pkq[0:64, :],
                                             mybir.ActivationFunctionType.Identity,
                                             bias=biask)
                        if h % 2 == 0:
                            nc.scalar.activation(ksq[64:128, :], pkq[0:64, :],
                                                 SQUARE, bias=biask)
                        else:
                            nc.vector.scalar_tensor_tensor(
                                ksq[64:128, :], pkq[0:64, :], biask,
                                ksq[0:64, :], ADD, MULT)
                        qsq = sqp.tile([128, T], f32r, tag="sq")
                        nc.vector.tensor_scalar_add(qsq[0:64, :],
                                                    pkq[64:128, :], biasq)
                        nc.vector.scalar_tensor_tensor(
                            qsq[64:128, :], pkq[64:128, :],
                            kqb_sb[0:64, H + h:H + h + 1],
                            qsq[0:64, :], ADD, MULT)
                        # prm exponent matmuls (lhsT = [w_h^T ; -0.5])
                        pk = ps_prm.tile([M, T], f32, tag="prm")
                        nc.tensor.matmul(pk, prmw_sb[:, h, :], ksq,
                                         start=True, stop=True)
                        pq = ps_prm.tile([M, T], f32, tag="prm")
                        nc.tensor.matmul(pq, prmw_sb[:, h, :], qsq,
                                         start=True, stop=True)
                        if gi == 0:
                            kpt = kptp.tile([128, T], bf16, tag="kpt")
                            kpts.append(kpt)
                        nc.scalar.activation(
                            kpts[g][32 * gi:32 * (gi + 1), :], pk, EXP)
                        nc.scalar.activation(
                            qp_pack[32 * gi:32 * (gi + 1), g, t0:t0 + T],
                            pq, EXP)

                    # ---- kp transpose + kptv accumulation ----
                    pkptv = ps_kptv.tile([128, 3, HD + 1], f32, tag="pkptv")
                    for g in range(3):
                        for j in range(4):
                            ptr = ps_tr.tile([128, 128], bf16, tag="ptr")
                            nc.tensor.transpose(
                                ptr, kpts[g][:, j * 128:(j + 1) * 128],
                                ident_sb)
                            kpn = kpnp.tile([128, 128], bf16, tag="kpn")
                            nc.vector.tensor_copy(kpn, ptr)
                            for gi in range(4):
                                h = g * 4 + gi
                                nc.tensor.matmul(
                                    pkptv[32 * gi:32 * (gi + 1), g, :],
                                    kpn[:, 32 * gi:32 * (gi + 1)],
                                    vaugs[j][:, h, :],
                                    start=(j == 0), stop=(j == 3),
                                    tile_position=(0, 32 * gi))
                    for g in range(3):
                        nc.vector.tensor_tensor(acc[:, g, :], pkptv[:, g, :],
                                                acc[:, g, :], ADD)

            # ================= PASS 2 =================
            for c in range(6):
                t_ = const.tile([128, C], f32r, tag=f"pwT{c}")
                nc.sync.dma_start(t_, pwT[c * 128:(c + 1) * 128, :].bitcast(f32r))
                pwT_sb.append(t_)
            pb_sb = const.tile([128, 6], f32, tag="pb")
            nc.sync.dma_start(pb_sb, pb[:])
            with tc.tile_pool(name="yw", bufs=1) as ywp, \
                 tc.tile_pool(name="rc", bufs=8) as rcp, \
                 tc.tile_pool(name="bc", bufs=10) as bcp, \
                 tc.tile_pool(name="ybig", bufs=10) as ybp, \
                 tc.tile_pool(name="so", bufs=3) as sop, \
                 tc.tile_pool(name="ps_y", bufs=5, space="PSUM") as ps_y, \
                 tc.tile_pool(name="ps_p", bufs=3, space="PSUM") as ps_p:

                padded = ywp.tile([128, H, HD + 1], bf16, tag="padded")
                nc.vector.memset(padded, 0.0)
                for h in range(12):
                    g, gi = h // 4, h % 4
                    nc.vector.tensor_copy(
                        padded[32 * gi:32 * (gi + 1), h, :],
                        acc[32 * gi:32 * (gi + 1), g, :])
                eps_sb = ywp.tile([1, 1], f32, tag="eps")
                nc.vector.memset(eps_sb, EPS_EFF)

                for it in range(NT):
                    t0 = it * T
                    ybcs = []
                    for h in range(12):
                        g = h // 4
                        psy = ps_y.tile([HD + 1, T], f32, tag="psy")
                        nc.tensor.matmul(psy, padded[:, h, :],
                                         qp_pack[:, g, t0:t0 + T],
                                         start=True, stop=True)
                        rc = rcp.tile([1, T], f32, tag="rc")
                        nc.scalar.activation(
                            rc, psy[64:65, :],
                            mybir.ActivationFunctionType.Identity,
                            bias=eps_sb[:])
                        nc.vector.reciprocal_approx_fast(out=rc, in_=rc)
                        bc = bcp.tile([64, T], f32, tag="bc")
                        nc.gpsimd.partition_broadcast(bc, rc)
                        ch, half = h // 2, h % 2
                        if half == 0:
                            ybc = ybp.tile([128, T], f32r, tag="ybig")
                            ybcs.append(ybc)
                        nc.vector.tensor_tensor(
                            ybcs[ch][64 * half:64 * (half + 1), :],
                            psy[0:64, :], bc, MULT)
                    for i2 in range(6):
                        psp = ps_p.tile([128, T], f32, tag="psp")
                        for cc in range(6):
                            c = (cc + i2) % 6
                            nc.tensor.matmul(
                                psp, pwT_sb[c][:, i2 * 128:(i2 + 1) * 128],
                                ybcs[c], start=(cc == 0), stop=(cc == 5))
                        so = sop.tile([128, T], f32, tag="so")
                        nc.scalar.activation(so, psp,
                                             mybir.ActivationFunctionType.Identity,
                                             bias=pb_sb[:, i2:i2 + 1])
                        nc.sync.dma_start(yT[i2 * 128:(i2 + 1) * 128,
                                             t0:t0 + T], so)

    nc.compile()
    return nc


def _prep_inputs(x, kqv_w, kqv_b, proj_w, proj_b, w):
    x = np.asarray(x, np.float32)
    kqv_w = np.asarray(kqv_w, np.float32)
    kqv_b = np.asarray(kqv_b, np.float32)
    proj_w = np.asarray(proj_w, np.float32)
    proj_b = np.asarray(proj_b, np.float32)
    w = np.asarray(w, np.float32)

    Wk, Wq, Wv = kqv_w[0:C], kqv_w[C:2 * C], kqv_w[2 * C:3 * C]
    wkq = np.empty((C, 2 * C), np.float32)
    for h in range(H):
        wkq[:, h * 128:h * 128 + 64] = Wk[h * 64:(h + 1) * 64, :].T
        wkq[:, h * 128 + 64:h * 128 + 128] = Wq[h * 64:(h + 1) * 64, :].T
    wv = np.ascontiguousarray(Wv.T)

    prmw = np.empty((128, H * M), np.float32)
    for h in range(H):
        prmw[0:64, h * M:(h + 1) * M] = w[h].T
    prmw[64:128, :] = -0.5

    kqb = np.zeros((128, 2 * H), np.float32)
    for h in range(H):
        kqb[0:64, h] = kqv_b[h * 64:(h + 1) * 64]
        kqb[64:128, h] = kqv_b[C + h * 64:C + (h + 1) * 64]
        kqb[0:64, H + h] = kqv_b[C + h * 64:C + (h + 1) * 64]
    vbr = np.broadcast_to(kqv_b[2 * C:3 * C], (128, C)).copy()

    pwT = np.ascontiguousarray(proj_w.T)
    pb = np.ascontiguousarray(proj_b.reshape(6, 128).T)
    identb = np.eye(128, dtype=ml_dtypes.bfloat16)

    shared = {"wkq": wkq, "wv": wv, "prmw": prmw, "kqb": kqb,
              "vbr": vbr, "pwT": pwT, "pb": pb, "identb": identb}
    xTb = np.ascontiguousarray(x.transpose(0, 2, 1))  # [B, C, N]
    return [dict(shared, xT=xTb[b]) for b in range(B)]


def kernel(x, kqv_w, kqv_b, proj_w, proj_b, w):
    global LAST_EXEC_NS
    from concourse.bass_utils import run_bass_kernel_spmd

    if "nc" not in _CACHE:
        _CACHE["nc"] = _build()
    nc = _CACHE["nc"]

    in_maps = _prep_inputs(x, kqv_w, kqv_b, proj_w, proj_b, w)
    res = run_bass_kernel_spmd(nc, in_maps, list(range(B)), trace=TRACE)
    LAST_EXEC_NS = res.exec_time_ns
    out = np.empty((B, N, C), np.float32)
    for b in range(B):
        out[b] = res.results[b]["yT"].T
    return out


# revision 40
# speedup vs baseline: 1.0127x; 1.0127x over previous
"""AttentionPerformer Trainium2 kernel.

Strategy: data-parallel over batch B=8 -> one NeuronCore per batch element.
Per core, everything is computed in a "feature-major" (transposed) layout so
that the big matmuls keep their stationary weights resident and the Performer
random-feature exponent (wtx - 0.5*|k|^2) comes directly out of the PE via a
fused contraction over [k ; k^2].

Math (per b, h; eps folded: dropping both 1/sqrt(M) factors rescales
numerator and denominator by M, so eps becomes M*eps):
    kq^T = W_kq^T @ x^T            (feature-major, per-head k/q stacked 64+64)
    e_k  = w_h @ k - 0.5|k|^2      (one PE pass over [k^T ; (k^T)^2])
    kp = exp(e_k), qp = exp(e_q)
    kptv[m, n] = sum_t kp[t,m] v[t,n] ;  kp_sum[m] = sum_t kp[t,m]
    y^T_h = (kptv_aug^T @ qp^T) ; D = kp_sum . qp
    out^T = proj_w^T.T @ (y^T / (D + M*eps)) + proj_b

dtypes: fp32r (TRN2 full-rate fp32, ~1.5e-4) on the exponent-critical path,
bf16 on the kp/v/qp value path (errors average over the M=32 contraction).
"""
import sys
sys.path.insert(0, '/opt/trn_rl_repo')

import numpy as np
import ml_dtypes

B, N, C = 8, 4096, 768
H, HD, M = 12, 64, 32
T = 512                 # t-tile size
NT = N // T             # 8 tiles
EPS_EFF = float(M) * 1e-8

_CACHE = {}
TRACE = False
LAST_EXEC_NS = None


def _build():
    import concourse.bass as bass
    import concourse.tile as tile
    from concourse import bacc, mybir

    f32 = mybir.dt.float32
    f32r = mybir.dt.float32r
    bf16 = mybir.dt.bfloat16
    ADD = mybir.AluOpType.add
    MULT = mybir.AluOpType.mult
    EXP = mybir.ActivationFunctionType.Exp
    SQUARE = mybir.ActivationFunctionType.Square
    RECIP = mybir.ActivationFunctionType.Reciprocal

    nc = bacc.Bacc()

    xT = nc.dram_tensor("xT", [C, N], f32, kind="ExternalInput")
    wkq = nc.dram_tensor("wkq", [C, 2 * C], f32, kind="ExternalInput")
    wv = nc.dram_tensor("wv", [C, C], f32, kind="ExternalInput")
    prmw = nc.dram_tensor("prmw", [128, H * M], f32, kind="ExternalInput")
    kqb = nc.dram_tensor("kqb", [128, 2 * H], f32, kind="ExternalInput")
    vbr = nc.dram_tensor("vbr", [128, C], f32, kind="ExternalInput")
    pwT = nc.dram_tensor("pwT", [C, C], f32, kind="ExternalInput")
    pb = nc.dram_tensor("pb", [128, 6], f32, kind="ExternalInput")
    identb = nc.dram_tensor("identb", [128, 128], mybir.dt.bfloat16,
                            kind="ExternalInput")
    yT = nc.dram_tensor("yT", [C, N], f32, kind="ExternalOutput")

    with tile.TileContext(nc) as tc:
        import contextlib
        with contextlib.ExitStack() as ctx:
            const = ctx.enter_context(tc.tile_pool(name="const", bufs=1))

            # ---- resident constants ----
            wkq_sb = []
            wv_sb = []
            pwT_sb = []
            for c in range(6):
                t_ = const.tile([128, 2 * C], f32r, tag=f"wkq{c}")
                nc.sync.dma_start(t_, wkq[c * 128:(c + 1) * 128, :].bitcast(f32r))
                wkq_sb.append(t_)
            for c in range(6):
                t_ = const.tile([128, C], f32r, tag=f"wv{c}")
                nc.sync.dma_start(t_, wv[c * 128:(c + 1) * 128, :].bitcast(f32r))
                wv_sb.append(t_)
            prmw_sb = const.tile([128, H, M], f32r, tag="prmw")
            nc.sync.dma_start(prmw_sb, prmw[:].bitcast(f32r).rearrange(
                "p (h m) -> p h m", h=H))
            kqb_sb = const.tile([128, 2 * H], f32, tag="kqb")
            nc.sync.dma_start(kqb_sb, kqb[:])
            vbr_sb = const.tile([128, C], f32, tag="vbr")
            nc.sync.dma_start(vbr_sb, vbr[:])
            ident_sb = const.tile([128, 128], bf16, tag="identb")
            nc.sync.dma_start(ident_sb, identb[:])

            # resident accumulators / state
            qp_pack = const.tile([128, 3, N], bf16, tag="qp_pack")
            acc = const.tile([128, 3, M * 2 + 1], f32, tag="acc")
            nc.vector.memset(acc, 0.0)
            # acc layout: [128, g, 65]; head h = 4g+gi lives at partitions
            # 32gi..32gi+32: cols 0:64 = kptv^T, col 64 = kp_sum

            # ================= PASS 1 =================
            with tc.tile_pool(name="xt", bufs=3) as xtp, \
                 tc.tile_pool(name="kq", bufs=6) as kqp, \
                 tc.tile_pool(name="sq", bufs=6) as sqp, \
                 tc.tile_pool(name="kpt", bufs=3) as kptp, \
                 tc.tile_pool(name="kpn", bufs=4) as kpnp, \
                 tc.tile_pool(name="vaug", bufs=6) as vap, \
                 tc.tile_pool(name="ps_kq", bufs=2, space="PSUM") as ps_kq, \
                 tc.tile_pool(name="ps_v", bufs=1, space="PSUM") as ps_v, \
                 tc.tile_pool(name="ps_prm", bufs=2, space="PSUM") as ps_prm, \
                 tc.tile_pool(name="ps_kptv", bufs=1, space="PSUM") as ps_kptv, \
                 tc.tile_pool(name="ps_tr", bufs=1, space="PSUM") as ps_tr:

                for it in range(NT):
                    t0 = it * T
                    xt = xtp.tile([128, 6, T], f32r, tag="xt")
                    for c in range(6):
                        nc.sync.dma_start(
                            xt[:, c, :],
                            xT[c * 128:(c + 1) * 128, t0:t0 + T].bitcast(f32r))

                    # ---- k/q head tiles, v-sub groups interleaved ----
                    vaugs = []

                    def emit_v(sub):
                        psv = ps_v.tile([128, C], f32, tag="psv")
                        for c in range(6):
                            lhs = xt[:, c, sub * 128:(sub + 1) * 128]
                            nc.tensor.matmul(psv[:, 0:512], lhs,
                                             wv_sb[c][:, 0:512],
                                             start=(c == 0), stop=(c == 5))
                            nc.tensor.matmul(psv[:, 512:768], lhs,
                                             wv_sb[c][:, 512:768],
                                             start=(c == 0), stop=(c == 5))
                        va = vap.tile([128, H, HD + 1], bf16, tag="vaug")
                        nc.vector.tensor_tensor(
                            va[:, :, 0:HD],
                            psv[:].rearrange("p (h n) -> p h n", h=H),
                            vbr_sb[:].rearrange("p (h n) -> p h n", h=H), ADD)
                        nc.vector.memset(va[:, :, HD:HD + 1], 1.0)
                        vaugs.append(va)

                    kpts = []
                    for h in range(12):
                        g, gi = h // 4, h % 4
                        if h % 3 == 0:
                            emit_v(h // 3)
                        pkq = ps_kq.tile([128, T], f32, tag="pkq")
                        for c in range(6):
                            nc.tensor.matmul(
                                pkq, wkq_sb[c][:, h * 128:(h + 1) * 128],
                                xt[:, c, :], start=(c == 0), stop=(c == 5))
                        biask = kqb_sb[0:64, h:h + 1]
                        biasq = kqb_sb[64:128, h:h + 1]
                        # ksq = [k+b ; (k+b)^2], qsq = [q+b ; (q+b)^2]
                        ksq = kqp.tile([128, T], f32r, tag="kq")
                        nc.scalar.activation(ksq[0:64, :], pkq[0:64, :],
                                             mybir.ActivationFunctionType.Identity,
                                             bias=biask)
                        if h % 2 == 0:
                            nc.scalar.activation(ksq[64:128, :], pkq[0:64, :],
                                                 SQUARE, bias=biask)
                        else:
                            nc.vector.scalar_tensor_tensor(
                                ksq[64:128, :], pkq[0:64, :], biask,
                                ksq[0:64, :], ADD, MULT)
                        qsq = sqp.tile([128, T], f32r, tag="sq")
                        nc.vector.tensor_scalar_add(qsq[0:64, :],
                                                    pkq[64:128, :], biasq)
                        nc.vector.scalar_tensor_tensor(
                            qsq[64:128, :], pkq[64:128, :],
                            kqb_sb[0:64, H + h:H + h + 1],
                            qsq[0:64, :], ADD, MULT)
                        # prm exponent matmuls (lhsT = [w_h^T ; -0.5])
                        pk = ps_prm.tile([M, T], f32, tag="prm")
                        nc.tensor.matmul(pk, prmw_sb[:, h, :], ksq,
                                         start=True, stop=True)
                        pq = ps_prm.tile([M, T], f32, tag="prm")
                        nc.tensor.matmul(pq, prmw_sb[:, h, :], qsq,
                                         start=True, stop=True)
                        if gi == 0:
                            kpt = kptp.tile([128, T], bf16, tag="kpt")
                            kpts.append(kpt)
                        nc.scalar.activation(
                            kpts[g][32 * gi:32 * (gi + 1), :], pk, EXP)
                        nc.scalar.activation(
                            qp_pack[32 * gi:32 * (gi + 1), g, t0:t0 + T],
                            pq, EXP)

                    # ---- kp transpose + kptv accumulation ----
                    pkptv = ps_kptv.tile([128, 3, HD + 1], f32, tag="pkptv")
                    for g in range(3):
                        for j in range(4):
                            ptr = ps_tr.tile([128, 128], bf16, tag="ptr")
                            nc.tensor.transpose(
                                ptr, kpts[g][:, j * 128:(j + 1) * 128],
                                ident_sb)
                            kpn = kpnp.tile([128, 128], bf16, tag="kpn")
                            nc.vector.tensor_copy(kpn, ptr)
                            for gi in range(4):
                                h = g * 4 + gi
                                nc.tensor.matmul(
                                    pkptv[32 * gi:32 * (gi + 1), g, :],
                                    kpn[:, 32 * gi:32 * (gi + 1)],
                                    vaugs[j][:, h, :],
                                    start=(j == 0), stop=(j == 3),
                                    tile_position=(0, 32 * gi))
                    for g in range(3):
                        nc.vector.tensor_tensor(acc[:, g, :], pkptv[:, g, :],
                                                acc[:, g, :], ADD)

            # ================= PASS 2 =================
            for c in range(6):
                t_ = const.tile([128, C], f32r, tag=f"pwT{c}")
                nc.sync.dma_start(t_, pwT[c * 128:(c + 1) * 128, :].bitcast(f32r))
                pwT_sb.append(t_)
            pb_sb = const.tile([128, 6], f32, tag="pb")
            nc.sync.dma_start(pb_sb, pb[:])
            with tc.tile_pool(name="yw", bufs=1) as ywp, \
                 tc.tile_pool(name="rc", bufs=8) as rcp, \
                 tc.tile_pool(name="bc", bufs=10) as bcp, \
                 tc.tile_pool(name="ybig", bufs=10) as ybp, \
                 tc.tile_pool(name="so", bufs=4) as sop, \
                 tc.tile_pool(name="ps_y", bufs=5, space="PSUM") as ps_y, \
                 tc.tile_pool(name="ps_p", bufs=3, space="PSUM") as ps_p:

                padded = ywp.tile([128, H, HD + 1], bf16, tag="padded")
                nc.vector.memset(padded, 0.0)
                for h in range(12):
                    g, gi = h // 4, h % 4
                    nc.vector.tensor_copy(
                        padded[32 * gi:32 * (gi + 1), h, :],
                        acc[32 * gi:32 * (gi + 1), g, :])
                eps_sb = ywp.tile([1, 1], f32, tag="eps")
                nc.vector.memset(eps_sb, EPS_EFF)

                for it in range(NT):
                    t0 = it * T
                    ybcs = []
                    for h in range(12):
                        g = h // 4
                        psy = ps_y.tile([HD + 1, T], f32, tag="psy")
                        nc.tensor.matmul(psy, padded[:, h, :],
                                         qp_pack[:, g, t0:t0 + T],
                                         start=True, stop=True)
                        rc = rcp.tile([1, T], f32, tag="rc")
                        nc.scalar.activation(
                            rc, psy[64:65, :],
                            mybir.ActivationFunctionType.Identity,
                            bias=eps_sb[:])
                        nc.vector.reciprocal_approx_fast(out=rc, in_=rc)
                        bc = bcp.tile([64, T], f32, tag="bc")
                        nc.gpsimd.partition_broadcast(bc, rc)
                        ch, half = h // 2, h % 2
                        if half == 0:
                            ybc = ybp.tile([128, T], f32r, tag="ybig")
                            ybcs.append(ybc)
                        nc.vector.tensor_tensor(
                            ybcs[ch][64 * half:64 * (half + 1), :],
                            psy[0:64, :], bc, MULT)
                    for i2 in range(6):
                        psp = ps_p.tile([128, T], f32, tag="psp")
                        for c in range(6):
                            nc.tensor.matmul(
                                psp, pwT_sb[c][:, i2 * 128:(i2 + 1) * 128],
                                ybcs[c], start=(c == 0), stop=(c == 5))
                        so = sop.tile([128, T], f32, tag="so")
                        nc.scalar.activation(so, psp,
                                             mybir.ActivationFunctionType.Identity,
                                             bias=pb_sb[:, i2:i2 + 1])
                        nc.sync.dma_start(yT[i2 * 128:(i2 + 1) * 128,
                                             t0:t0 + T], so)

    nc.compile()
    return nc


def _prep_inputs(x, kqv_w, kqv_b, proj_w, proj_b, w):
    x = np.asarray(x, np.float32)
    kqv_w = np.asarray(kqv_w, np.float32)
    kqv_b = np.asarray(kqv_b, np.float32)
    proj_w = np.asarray(proj_w, np.float32)
    proj_b = np.asarray(proj_b, np.float32)
    w = np.asarray(w, np.float32)

    Wk, Wq, Wv = kqv_w[0:C], kqv_w[C:2 * C], kqv_w[2 * C:3 * C]
    wkq = np.empty((C, 2 * C), np.float32)
    for h in range(H):
        wkq[:, h * 128:h * 128 + 64] = Wk[h * 64:(h + 1) * 64, :].T
        wkq[:, h * 128 + 64:h * 128 + 128] = Wq[h * 64:(h + 1) * 64, :].T
    wv = np.ascontiguousarray(Wv.T)

    prmw = np.empty((128, H * M), np.float32)
    for h in range(H):
        prmw[0:64, h * M:(h + 1) * M] = w[h].T
    prmw[64:128, :] = -0.5

    kqb = np.zeros((128, 2 * H), np.float32)
    for h in range(H):
        kqb[0:64, h] = kqv_b[h * 64:(h + 1) * 64]
        kqb[64:128, h] = kqv_b[C + h * 64:C + (h + 1) * 64]
        kqb[0:64, H + h] = kqv_b[C + h * 64:C + (h + 1) * 64]
    vbr = np.broadcast_to(kqv_b[2 * C:3 * C], (128, C)).copy()

    pwT = np.ascontiguousarray(proj_w.T)
    pb = np.ascontiguousarray(proj_b.reshape(6, 128).T)
    identb = np.eye(128, dtype=ml_dtypes.bfloat16)

    shared = {"wkq": wkq, "wv": wv, "prmw": prmw, "kqb": kqb,
              "vbr": vbr, "pwT": pwT, "pb": pb, "identb": identb}
    xTb = np.ascontiguousarray(x.transpose(0, 2, 1))  # [B, C, N]
    return [dict(shared, xT=xTb[b]) for b in range(B)]


def kernel(x, kqv_w, kqv_b, proj_w, proj_b, w):
    global LAST_EXEC_NS
    from concourse.bass_utils import run_bass_kernel_spmd

    if "nc" not in _CACHE:
        _CACHE["nc"] = _build()
    nc = _CACHE["nc"]

    in_maps = _prep_inputs(x, kqv_w, kqv_b, proj_w, proj_b, w)
    res = run_bass_kernel_spmd(nc, in_maps, list(range(B)), trace=TRACE)
    LAST_EXEC_NS = res.exec_time_ns
    out = np.empty((B, N, C), np.float32)
    for b in range(B):
        out[b] = res.results[b]["yT"].T
    return out
